# revision 28
# baseline (speedup 1.0000x reference)
"""Multi-head attention (B=1, S=4096, D=768, H=12, Hd=64) on 8 trn2 cores.

Sharding: 4 head-groups (3 heads = 192 dims, Megatron column-split wq/wk/wv,
row-split ww) x 2 query-chunks (2048 rows).  core = g*2 + c.
Each core returns a partial output [2048, 768]; host sums the 4 group
partials per chunk and adds (bv @ ww.T + bw).

Per-core plan (all matmul inputs float32r = full-rate fp32, and every
matmul is shaped to full K=128 contraction / M=128 stationary columns --
K<128 or M<128 matmuls run at ~half rate on this hardware):
  xT  [768, 4096]   x transposed (keys/values source), xqT = query columns.
  KT  [128, 2, S]   transposed K: pair 0 = heads 0,1 (dims 0-127); pair 1
                    rows 0-63 = head 2, rows 64-127 = 0.
  QTz [128, 3, SQ]  per-head zero-padded Q^T: scores matmuls contract the
                    full 128 rows of a KT pair; zeros in the complementary
                    half kill the other head's term.
  V2  [128, 3, NJ*65+63]  flat per-head V: cols j*65..j*65+63 = V rows,
                    col j*65+64 = 1.0; 63-col tail so the M=128 attnV lhsT
                    AP may overrun harmlessly (garbage lands in unused psum
                    partitions 65..127).  attnV accumulates out^T (rows
                    0-63) and the softmax denominator (row 64) over key
                    tiles in PSUM.
  scoresT psum [128 keys, 512 q]; exp on ACT engine psum->SBUF (x1/8
                    folded into the activation scale; no max subtraction:
                    |scores| < 3 for this input distribution).
  y6  [128, 3, SQ]  normalized attn out^T per head (rows 64-127 zeroed);
                    out-proj contracts 3 x 64 as full-K matmuls against a
                    zero-padded ww6.
"""

import sys

if "/opt/trn_rl_repo" not in sys.path:
    sys.path.insert(0, "/opt/trn_rl_repo")

import os

import numpy as np

import concourse.bacc as bacc
import concourse.mybir as mybir
import concourse.tile as tile
from concourse.bass_utils import run_bass_kernel_spmd
from concourse.vector_clock import ScopedClock

F32 = mybir.dt.float32
F32R = mybir.dt.float32r
BF16 = mybir.dt.bfloat16
MD = {"f32r": F32R, "bf16": BF16, "f32": F32}[os.environ.get("MM_DTYPE", "f32r")]

S = 4096          # sequence length
D = 768           # model dim
NG = 4            # head groups (cores axis 1)
NC = 2            # query chunks (cores axis 2)
DH = D // NG      # dims per group = 192
NH = 3            # heads per group
SQ = S // NC      # queries per core = 2048
KO = D // 128     # contraction subtiles = 6
NJ = S // 128     # key tiles = 32
AF = mybir.ActivationFunctionType
SCALE = 0.125     # 1/sqrt(64)
CHUNKS = [3] * 10 + [2]   # 32 key tiles in exp-sized chunks
PAIRS = (0, 0, 1)  # KT pair index per head

_PATCHED = False


def _patch_drain():
    """walrus in this container rejects >1 sync-wait per instruction
    ("Too many sync wait commands").  TileContext's tail drain aggregates one
    wait per live tile semaphore; redistribute them one-per-nop.  (Bacc's
    generate_event_semaphores handles the rest of the kernel.)"""
    global _PATCHED
    if _PATCHED:
        return
    _PATCHED = True

    def _drain_and_barrier(self, tick_clock, wait_clock):
        nc = self.nc
        drain_inst = nc.sync.drain()
        wait_clock.add_sem_waits(
            drain_inst.ins, ScopedClock({None: tick_clock.global_clock})
        )
        si = drain_inst.ins.sync_info
        waits = list(si.on_wait) if si is not None else []
        if len(waits) > 1:
            drain_inst.ins.sync_info = mybir.SyncInfo(
                on_wait=[waits[0]], on_update=list(si.on_update)
            )
            for w in waits[1:]:
                nop = nc.sync.nop(nofuse=True)
                nop.ins.sync_info = mybir.SyncInfo(on_wait=[w], on_update=[])
        nc.all_engine_barrier()
        assert self.sems is not None
        popped = nc._tile_sem_poison_stack.pop()
        assert popped is self._sem_poison
        nc.clear_and_free_semaphores(list(self.sems.allocated().values()))
        nc.all_engine_barrier()

    tile.TileContext._drain_and_barrier = _drain_and_barrier


def build_nc(loop_n=None):
    _patch_drain()
    nc = bacc.Bacc("TRN2", target_bir_lowering=False)

    xT = nc.dram_tensor("xT", [D, S], MD, kind="ExternalInput")
    xqT = nc.dram_tensor("xqT", [D, SQ], MD, kind="ExternalInput")
    wqT = nc.dram_tensor("wqT", [D, DH], MD, kind="ExternalInput")
    wkT = nc.dram_tensor("wkT", [D, DH], MD, kind="ExternalInput")
    wvT = nc.dram_tensor("wvT", [D, DH], MD, kind="ExternalInput")
    wwT = nc.dram_tensor("wwT", [DH, D], MD, kind="ExternalInput")
    bq = nc.dram_tensor("bq", [128, 2], F32, kind="ExternalInput")
    bk = nc.dram_tensor("bk", [128, 2], F32, kind="ExternalInput")
    out = nc.dram_tensor("out", [SQ, D], F32, kind="ExternalOutput")

    xT_r = xT.rearrange("(ko p) n -> p ko n", p=128)
    xqT_r = xqT.rearrange("(ko p) n -> p ko n", p=128)
    wqT_r = wqT.rearrange("(ko p) m -> p ko m", p=128)
    wkT_r = wkT.rearrange("(ko p) m -> p ko m", p=128)
    wvT_r = wvT.rearrange("(ko p) m -> p ko m", p=128)
    ww6_r = wwT.rearrange("(h l) o -> l h o", l=64)   # [64, 3, 768]

    with tile.TileContext(nc) as tc:
        import contextlib

        with contextlib.ExitStack() as ctx:
            if loop_n is not None:
                ctx.enter_context(tc.For_i(0, loop_n, 1))
            persist = ctx.enter_context(tc.tile_pool(name="persist", bufs=1))
            KT = persist.tile([128, 2, S], MD)               # 32KB/part
            V2 = persist.tile([128, NH, NJ * 65 + 63], MD)   # 25.7KB/part
            QTz = persist.tile([128, NH, SQ], MD)            # 24KB/part
            ones_f32 = persist.tile([128, 1], F32)
            zero_f32 = persist.tile([128, 1], F32)
            nc.vector.memset(ones_f32[:], 1.0)
            nc.vector.memset(zero_f32[:], 0.0)
            for h in range(NH):
                v2h = V2[:, h, 0:NJ * 65].rearrange("l (j c) -> l j c", c=65)
                nc.vector.tensor_copy(
                    v2h[:, :, 64:65],
                    ones_f32[:, 0:1].to_broadcast((128, NJ, 1)),
                )
                nc.vector.tensor_copy(
                    V2[:, h, NJ * 65:],
                    zero_f32[:, 0:1].to_broadcast((128, 63)),
                )
                # zero the complementary contraction half of QTz
                z0, z1 = (64, 128) if h % 2 == 0 else (0, 64)
                nc.vector.tensor_copy(
                    QTz[z0:z1, h, :],
                    zero_f32[z0:z1, 0:1].to_broadcast((64, SQ)),
                )

            with tc.tile_pool(name="proj", bufs=1) as proj, \
                 tc.tile_pool(name="ps12", bufs=4, space="PSUM") as ps12:
                # weights padded 192 -> 256 cols (zeros) so the second
                # m-tile / the V-proj rhs still run as full-width matmuls
                wk_sb = proj.tile([128, KO, 256], MD)
                wv_sb = proj.tile([128, KO, 256], MD)
                wq_sb = proj.tile([128, KO, 256], MD)
                xq_sb = proj.tile([128, KO, SQ], MD)
                bq_sb = proj.tile([128, 2], F32)
                bk_sb = proj.tile([128, 2], F32)
                for w_sb in (wk_sb, wv_sb, wq_sb):
                    nc.vector.tensor_copy(
                        w_sb[:, :, DH:],
                        zero_f32[:, 0:1].to_broadcast((128, KO, 256 - DH)),
                    )
                nc.sync.dma_start(wk_sb[:, :, 0:DH], wkT_r[:])
                nc.sync.dma_start(bk_sb[:], bk[:])
                nc.sync.dma_start(wv_sb[:, :, 0:DH], wvT_r[:])

                # ------------- phase 1: K/V projections (stream xT) ------
                with tc.tile_pool(name="xstream", bufs=3) as xs_pool:
                    for n in range(S // 512):
                        ns = slice(n * 512, (n + 1) * 512)
                        xb = xs_pool.tile([128, KO, 512], MD, tag="xb")
                        nc.sync.dma_start(xb[:], xT_r[:, :, ns])
                        for m in range(2):
                            ps = ps12.tile([128, 512], F32, tag="qk")
                            for ko in range(KO):
                                nc.tensor.matmul(
                                    ps[:],
                                    wk_sb[:, ko, m * 128:(m + 1) * 128],
                                    xb[:, ko, :],
                                    start=(ko == 0), stop=(ko == KO - 1),
                                )
                            nc.vector.tensor_scalar_add(
                                KT[:, m, ns], ps[:], bk_sb[:, m:m + 1],
                            )
                        for j4 in range(4):
                            j = n * 4 + j4
                            ps = ps12.tile([128, 512], F32, tag="v")
                            for ko in range(KO):
                                nc.tensor.matmul(
                                    ps[:, 0:256],
                                    xb[:, ko, j4 * 128:(j4 + 1) * 128],
                                    wv_sb[:, ko, :],
                                    start=(ko == 0), stop=(ko == KO - 1),
                                )
                            for h in range(NH):
                                nc.vector.tensor_copy(
                                    V2[:, h, j * 65:j * 65 + 64],
                                    ps[:, h * 64:(h + 1) * 64],
                                )
                        if n == 0:
                            # deferred so they don't delay the first x block
                            nc.sync.dma_start(wq_sb[:, :, 0:DH], wqT_r[:])
                            nc.sync.dma_start(xq_sb[:], xqT_r[:])
                            nc.sync.dma_start(bq_sb[:], bq[:])

                # ---------------- phase 2: Q projection -> QTz -----------
                for m in range(2):
                    for n in range(SQ // 512):
                        ns = slice(n * 512, (n + 1) * 512)
                        ps = ps12.tile([128, 512], F32, tag="qk")
                        for ko in range(KO):
                            nc.tensor.matmul(
                                ps[:],
                                wq_sb[:, ko, m * 128:(m + 1) * 128],
                                xq_sb[:, ko, ns],
                                start=(ko == 0), stop=(ko == KO - 1),
                            )
                        if m == 0:
                            nc.vector.tensor_scalar_add(
                                QTz[0:64, 0, ns], ps[0:64, :],
                                bq_sb[0:64, 0:1],
                            )
                            nc.vector.tensor_scalar_add(
                                QTz[64:128, 1, ns], ps[64:128, :],
                                bq_sb[64:128, 0:1],
                            )
                        else:
                            nc.vector.tensor_scalar_add(
                                QTz[0:64, 2, ns], ps[0:64, :],
                                bq_sb[0:64, 1:2],
                            )

            # ---------------- phases 3+4 ----------------
            with tc.tile_pool(name="late", bufs=1) as late, \
                 tc.tile_pool(name="pt", bufs=3) as pt_pool, \
                 tc.tile_pool(name="dn", bufs=2) as dn_pool, \
                 tc.tile_pool(name="bc", bufs=2) as bc_pool, \
                 tc.tile_pool(name="ob", bufs=2) as ob_pool, \
                 tc.tile_pool(name="ps_sc", bufs=1, space="PSUM") as ps_sc, \
                 tc.tile_pool(name="ps_out", bufs=1, space="PSUM") as ps_out:
                # [128, ...] with zeroed rows 64-127: full-K out-proj
                y6 = late.tile([128, NH, SQ], MD)      # 24KB/part
                ww6 = late.tile([128, NH, D], MD)      # 9KB/part
                nc.sync.dma_start(ww6[0:64, :, :], ww6_r[:])
                nc.vector.tensor_copy(
                    y6[64:128, :, :].rearrange("l h q -> l (h q)"),
                    zero_f32[64:128, 0:1].to_broadcast((64, NH * SQ)),
                )
                nc.vector.tensor_copy(
                    ww6[64:128, :, :].rearrange("l h o -> l (h o)"),
                    zero_f32[64:128, 0:1].to_broadcast((64, NH * D)),
                )

                def attend(heads, qs):
                    """attention for 1 or 2 heads over one 512-query block"""
                    o_ps = {}
                    for tag, h in zip(("outA", "outB"), heads):
                        o_ps[h] = ps_out.tile([128, 512], F32, tag=tag, name=tag)
                    j0 = 0
                    for cs in CHUNKS:
                        scs = {}
                        for tag, h in zip(("scA", "scB"), heads):
                            scs[h] = ps_sc.tile([128, 3, 512], F32, tag=tag, name=tag)
                        for t in range(cs):
                            j = j0 + t
                            js = slice(j * 128, (j + 1) * 128)
                            for h in heads:
                                nc.tensor.matmul(
                                    scs[h][:, t, :],
                                    KT[:, PAIRS[h], js], QTz[:, h, qs],
                                    start=True, stop=True,
                                )
                        pts = {}
                        for tag, h in zip(("ptA", "ptB"), heads):
                            pts[h] = pt_pool.tile([128, 3, 512], MD, tag=tag, name=tag)
                            nc.scalar.activation(
                                pts[h][:, :cs, :], scs[h][:, :cs, :],
                                AF.Exp, scale=SCALE,
                            )
                        for t in range(cs):
                            j = j0 + t
                            for h in heads:
                                nc.tensor.matmul(
                                    o_ps[h][:, :],
                                    V2[:, h, j * 65:j * 65 + 128],
                                    pts[h][:, t, :],
                                    start=(j == 0), stop=(j == NJ - 1),
                                )
                        j0 += cs
                    # normalize: psum row 64 holds the softmax denominator
                    for h in heads:
                        dn = dn_pool.tile([1, 512], F32, tag="dn")
                        nc.vector.tensor_copy(dn[:], o_ps[h][64:65, :])
                        bc = bc_pool.tile([64, 512], F32, tag="bc")
                        nc.gpsimd.partition_broadcast(bc[:], dn[:], channels=64)
                        nc.vector.reciprocal(bc[:], bc[:])
                        nc.vector.tensor_mul(
                            y6[0:64, h, qs], o_ps[h][0:64, :], bc[:]
                        )

                for qh in range(SQ // 512):
                    qs = slice(qh * 512, (qh + 1) * 512)
                    attend((0, 1), qs)
                    attend((2,), qs)

                    # ---------- phase 4: out-projection for this q-half ----
                    for m in range(qh * 4, (qh + 1) * 4):
                        ms = slice(m * 128, (m + 1) * 128)
                        ob = ob_pool.tile([128, D], F32, tag="ob")
                        for n0, nw in ((0, 512), (512, 256)):
                            ps = ps_out.tile([128, 512], F32, tag="outA")
                            for h in range(NH):
                                nc.tensor.matmul(
                                    ps[:, :nw],
                                    y6[:, h, ms],
                                    ww6[:, h, n0:n0 + nw],
                                    start=(h == 0), stop=(h == NH - 1),
                                )
                            nc.vector.tensor_copy(ob[:, n0:n0 + nw], ps[:, :nw])
                        nc.sync.dma_start(out[ms, :], ob[:])

    nc.finalize()  # Bacc.compile(): reg alloc + split multi-sem-waits
    return nc


_NC_CACHE = None


def make_in_maps(x, wq, bq, wk, bk, wv, ww):
    npdt = mybir.dt.np(MD)
    x = np.ascontiguousarray(np.asarray(x, dtype=np.float32))
    xT_full = np.ascontiguousarray(x[0].T).astype(npdt)  # [D, S]

    def bias2(b):
        # [192] -> [128, 2]: col 0 = dims 0-127, col 1 = dims 128-191 + zeros
        o = np.zeros((128, 2), dtype=np.float32)
        o[:, 0] = b[0:128]
        o[0:64, 1] = b[128:192]
        return o

    in_maps = []
    for core in range(8):
        g, c = core // NC, core % NC
        gs = slice(g * DH, (g + 1) * DH)
        in_maps.append({
            "xT": xT_full,
            "xqT": np.ascontiguousarray(xT_full[:, c * SQ:(c + 1) * SQ]),
            "wqT": np.ascontiguousarray(wq[gs, :].T).astype(npdt),
            "wkT": np.ascontiguousarray(wk[gs, :].T).astype(npdt),
            "wvT": np.ascontiguousarray(wv[gs, :].T).astype(npdt),
            "wwT": np.ascontiguousarray(ww[:, gs].T).astype(npdt),
            "bq": bias2(bq[gs]),
            "bk": bias2(bk[gs]),
        })
    return in_maps


def kernel(x, wq, bq, wk, bk, wv, bv, ww, bw):
    global _NC_CACHE
    if _NC_CACHE is None:
        _NC_CACHE = build_nc()
    nc = _NC_CACHE

    in_maps = make_in_maps(x, wq, bq, wk, bk, wv, ww)
    res = run_bass_kernel_spmd(nc, in_maps, core_ids=list(range(8)))

    const_row = (bv @ ww.T + bw).astype(np.float32)  # [768]
    out = np.empty((1, S, D), dtype=np.float32)
    for c in range(NC):
        acc = res.results[0 * NC + c]["out"].copy()
        for g in range(1, NG):
            acc += res.results[g * NC + c]["out"]
        out[0, c * SQ:(c + 1) * SQ, :] = acc + const_row
    return out


# revision 29
# speedup vs baseline: 1.1828x; 1.1828x over previous
"""Multi-head attention (B=1, S=4096, D=768, H=12, Hd=64) on 8 trn2 cores.

Sharding: 2 head-groups (6 heads = 384 dims, Megatron column-split wq/wk/wv,
row-split ww) x 4 query-chunks (1024 rows).  core = g*4 + c.
Each core returns a partial output [1024, 768]; host sums the 2 group
partials per chunk and adds (bv @ ww.T + bw).

Per-core plan:
  xT  [768, 4096]   x transposed (keys/values source), xqT = query columns.
  QT/KT [128, 3, *] head-pair-packed transposed projections: partition
                    l, pair p -> local dim p*128+l.  The two heads of a pair
                    run their scores matmuls concurrently in the PE array via
                    contraction row-packing (base partitions 0 / 64).
  V2  [128, 32, 6, 65]  value rows (key j on partitions) per head, with a
                    ones column at index 64: the attnV matmul (M=65) then
                    accumulates both out^T (rows 0-63) and the softmax
                    denominator (row 64) over key tiles in PSUM.
  scoresT psum [128 keys, 512 q]; exp on ACT engine psum->SBUF (x1/8 folded
                    into the activation scale; no max subtraction needed:
                    |scores| < 3).
  y6  [64, 6, 1024] normalized attn output^T per head (64 partitions), so
                    no partition shifts are needed; out-proj contracts 6x64.
All matmul inputs are float32r (full-rate fp32 mode, moving dim >= 256).
"""

import sys

if "/opt/trn_rl_repo" not in sys.path:
    sys.path.insert(0, "/opt/trn_rl_repo")

import numpy as np

import concourse.bacc as bacc
import concourse.bass as bass
import concourse.mybir as mybir
import concourse.tile as tile
from concourse.bass_utils import run_bass_kernel_spmd
from concourse.vector_clock import ScopedClock

F32 = mybir.dt.float32
F32R = mybir.dt.float32r
BF16 = mybir.dt.bfloat16
import os
MD = {"f32r": F32R, "bf16": BF16, "f32": F32}[os.environ.get("MM_DTYPE", "f32r")]
MERGED_EXP = os.environ.get("MERGED_EXP", "1") == "1"

S = 4096          # sequence length
D = 768           # model dim
NG = 2            # head groups (cores axis 1)
NC = 4            # query chunks (cores axis 2)
DH = D // NG      # dims per group = 384
NP = DH // 128    # head pairs per group = 3
NH = 2 * NP       # heads per group = 6
SQ = S // NC      # queries per core = 1024
KO = D // 128     # contraction subtiles = 6
NJ = S // 128     # key tiles = 32
AF = mybir.ActivationFunctionType
SCALE = 0.125     # 1/sqrt(64)
CHUNKS = [3] * 10 + [2]   # 32 key tiles in exp-sized chunks

_PATCHED = False


def _patch_drain():
    """walrus in this container rejects >1 sync-wait per instruction
    ("Too many sync wait commands").  TileContext's tail drain aggregates one
    wait per live tile semaphore; redistribute them one-per-nop.  (Bacc's
    generate_event_semaphores handles the rest of the kernel.)"""
    global _PATCHED
    if _PATCHED:
        return
    _PATCHED = True

    def _drain_and_barrier(self, tick_clock, wait_clock):
        nc = self.nc
        drain_inst = nc.sync.drain()
        wait_clock.add_sem_waits(
            drain_inst.ins, ScopedClock({None: tick_clock.global_clock})
        )
        si = drain_inst.ins.sync_info
        waits = list(si.on_wait) if si is not None else []
        if len(waits) > 1:
            drain_inst.ins.sync_info = mybir.SyncInfo(
                on_wait=[waits[0]], on_update=list(si.on_update)
            )
            for w in waits[1:]:
                nop = nc.sync.nop(nofuse=True)
                nop.ins.sync_info = mybir.SyncInfo(on_wait=[w], on_update=[])
        nc.all_engine_barrier()
        assert self.sems is not None
        popped = nc._tile_sem_poison_stack.pop()
        assert popped is self._sem_poison
        nc.clear_and_free_semaphores(list(self.sems.allocated().values()))
        nc.all_engine_barrier()

    tile.TileContext._drain_and_barrier = _drain_and_barrier


def build_nc(loop_n=None):
    _patch_drain()
    nc = bacc.Bacc("TRN2", target_bir_lowering=False)

    xT = nc.dram_tensor("xT", [D, S], MD, kind="ExternalInput")
    xqT = nc.dram_tensor("xqT", [D, SQ], MD, kind="ExternalInput")
    wqT = nc.dram_tensor("wqT", [D, DH], MD, kind="ExternalInput")
    wkT = nc.dram_tensor("wkT", [D, DH], MD, kind="ExternalInput")
    wvT = nc.dram_tensor("wvT", [D, DH], MD, kind="ExternalInput")
    wwT = nc.dram_tensor("wwT", [DH, D], MD, kind="ExternalInput")
    bq = nc.dram_tensor("bq", [128, NP], F32, kind="ExternalInput")
    bk = nc.dram_tensor("bk", [128, NP], F32, kind="ExternalInput")
    out = nc.dram_tensor("out", [SQ, D], F32, kind="ExternalOutput")

    xT_r = xT.rearrange("(ko p) n -> p ko n", p=128)
    xqT_r = xqT.rearrange("(ko p) n -> p ko n", p=128)
    wqT_r = wqT.rearrange("(ko p) m -> p ko m", p=128)
    wkT_r = wkT.rearrange("(ko p) m -> p ko m", p=128)
    wvT_r = wvT.rearrange("(ko p) m -> p ko m", p=128)
    ww6_r = wwT.rearrange("(h l) o -> l h o", l=64)   # [64, 6, 768]

    with tile.TileContext(nc) as tc:
        import contextlib

        with contextlib.ExitStack() as ctx:
            if loop_n is not None:
                ctx.enter_context(tc.For_i(0, loop_n, 1))
            persist = ctx.enter_context(tc.tile_pool(name="persist", bufs=1))
            KT = persist.tile([128, NP, S], MD)        # 48KB/part
            # flat per-head V: cols j*65..j*65+63 = V rows, col j*65+64 = 1.0;
            # 63-col tail so the M=128 attnV lhsT AP may overrun harmlessly.
            V2 = persist.tile([128, NH, NJ * 65 + 63], MD)   # 51.4KB/part
            # per-head zero-padded Q^T: full-K(128) scores matmuls read the
            # whole head pair as lhsT; zeros in the complementary half kill
            # the cross-head term.  (K<128 matmuls run at half rate.)
            QTz = persist.tile([128, NH, SQ], MD)      # 24KB/part
            ones_f32 = persist.tile([128, 1], F32)
            zero_f32 = persist.tile([128, 1], F32)
            nc.vector.memset(ones_f32[:], 1.0)
            nc.vector.memset(zero_f32[:], 0.0)
            for h in range(NH):
                v2h = V2[:, h, 0:NJ * 65].rearrange("l (j c) -> l j c", c=65)
                nc.vector.tensor_copy(
                    v2h[:, :, 64:65],
                    ones_f32[:, 0:1].to_broadcast((128, NJ, 1)),
                )
                nc.vector.tensor_copy(
                    V2[:, h, NJ * 65:],
                    zero_f32[:, 0:1].to_broadcast((128, 63)),
                )
                # zero the complementary contraction half of QTz
                if h % 2 == 0:
                    nc.vector.tensor_copy(
                        QTz[64:128, h, :],
                        zero_f32[64:128, 0:1].to_broadcast((64, SQ)),
                    )
                else:
                    nc.vector.tensor_copy(
                        QTz[0:64, h, :],
                        zero_f32[0:64, 0:1].to_broadcast((64, SQ)),
                    )

            with tc.tile_pool(name="proj", bufs=1) as proj, \
                 tc.tile_pool(name="ps12", bufs=4, space="PSUM") as ps12:
                wk_sb = proj.tile([128, KO, DH], MD)
                wv_sb = proj.tile([128, KO, DH], MD)
                wq_sb = proj.tile([128, KO, DH], MD)
                xq_sb = proj.tile([128, KO, SQ], MD)
                bq_sb = proj.tile([128, NP], F32)
                bk_sb = proj.tile([128, NP], F32)
                nc.sync.dma_start(wk_sb[:], wkT_r[:])
                nc.sync.dma_start(bk_sb[:], bk[:])
                nc.sync.dma_start(wv_sb[:], wvT_r[:])

                # ------------- phase 1: K/V projections (stream xT) ------
                with tc.tile_pool(name="xstream", bufs=2) as xs_pool:
                    for n in range(S // 512):
                        xb = xs_pool.tile([128, KO, 512], MD, tag="xb")
                        nc.sync.dma_start(xb[:], xT_r[:, :, n * 512:(n + 1) * 512])
                        for p in range(NP):
                            ps = ps12.tile([128, 512], F32, tag="qk")
                            for ko in range(KO):
                                nc.tensor.matmul(
                                    ps[:],
                                    wk_sb[:, ko, p * 128:(p + 1) * 128],
                                    xb[:, ko, :],
                                    start=(ko == 0), stop=(ko == KO - 1),
                                )
                            nc.vector.tensor_scalar_add(
                                KT[:, p, n * 512:(n + 1) * 512], ps[:],
                                bk_sb[:, p:p + 1],
                            )
                        for j4 in range(4):
                            j = n * 4 + j4
                            ps = ps12.tile([128, 512], F32, tag="v")
                            for ko in range(KO):
                                nc.tensor.matmul(
                                    ps[:, :DH],
                                    xb[:, ko, j4 * 128:(j4 + 1) * 128],
                                    wv_sb[:, ko, :],
                                    start=(ko == 0), stop=(ko == KO - 1),
                                )
                            for h in range(NH):
                                nc.vector.tensor_copy(
                                    V2[:, h, j * 65:j * 65 + 64],
                                    ps[:, h * 64:(h + 1) * 64],
                                )
                        if n == 0:
                            # deferred so they don't delay the first x block
                            nc.sync.dma_start(wq_sb[:], wqT_r[:])
                            nc.sync.dma_start(xq_sb[:], xqT_r[:])
                            nc.sync.dma_start(bq_sb[:], bq[:])

                # ---------------- phase 2: Q projection -> QTz -----------
                for p in range(NP):
                    for n in range(SQ // 512):
                        ns = slice(n * 512, (n + 1) * 512)
                        ps = ps12.tile([128, 512], F32, tag="qk")
                        for ko in range(KO):
                            nc.tensor.matmul(
                                ps[:],
                                wq_sb[:, ko, p * 128:(p + 1) * 128],
                                xq_sb[:, ko, ns],
                                start=(ko == 0), stop=(ko == KO - 1),
                            )
                        nc.vector.tensor_scalar_add(
                            QTz[0:64, 2 * p, ns], ps[0:64, :],
                            bq_sb[0:64, p:p + 1],
                        )
                        nc.vector.tensor_scalar_add(
                            QTz[64:128, 2 * p + 1, ns], ps[64:128, :],
                            bq_sb[64:128, p:p + 1],
                        )

            # ---------------- phases 3+4 ----------------
            with tc.tile_pool(name="late", bufs=1) as late, \
                 tc.tile_pool(name="pt", bufs=2) as pt_pool, \
                 tc.tile_pool(name="dn", bufs=2) as dn_pool, \
                 tc.tile_pool(name="bc", bufs=2) as bc_pool, \
                 tc.tile_pool(name="ob", bufs=2) as ob_pool, \
                 tc.tile_pool(name="ps_sc", bufs=1, space="PSUM") as ps_sc, \
                 tc.tile_pool(name="ps_out", bufs=1, space="PSUM") as ps_out:
                # [128, ...] with zeroed rows 64-127: full-K out-proj.
                # ([64, x] tiles reserve the same per-partition bytes anyway.)
                y6 = late.tile([128, NH, SQ], MD)      # 24KB/part
                ww6 = late.tile([128, NH, D], MD)      # 18KB/part
                nc.sync.dma_start(ww6[0:64, :, :], ww6_r[:])
                nc.vector.tensor_copy(
                    y6[64:128, :, :].rearrange("l h q -> l (h q)"),
                    zero_f32[64:128, 0:1].to_broadcast((64, NH * SQ)),
                )
                nc.vector.tensor_copy(
                    ww6[64:128, :, :].rearrange("l h o -> l (h o)"),
                    zero_f32[64:128, 0:1].to_broadcast((64, NH * D)),
                )

                for qh in range(SQ // 512):
                    for p in range(NP):
                        qs = slice(qh * 512, (qh + 1) * 512)
                        oA = ps_out.tile([128, 512], F32, tag="outA")
                        oB = ps_out.tile([128, 512], F32, tag="outB")
                        j0 = 0
                        for cs in CHUNKS:
                            scA = ps_sc.tile([128, 3, 512], F32, tag="scA")
                            scB = ps_sc.tile([128, 3, 512], F32, tag="scB")
                            for t in range(cs):
                                j = j0 + t
                                js = slice(j * 128, (j + 1) * 128)
                                nc.tensor.matmul(
                                    scA[:, t, :],
                                    KT[:, p, js], QTz[:, 2 * p, qs],
                                    start=True, stop=True,
                                )
                                nc.tensor.matmul(
                                    scB[:, t, :],
                                    KT[:, p, js], QTz[:, 2 * p + 1, qs],
                                    start=True, stop=True,
                                )
                            ptA = pt_pool.tile([128, 3, 512], MD, tag="ptA")
                            ptB = pt_pool.tile([128, 3, 512], MD, tag="ptB")
                            nc.scalar.activation(
                                ptA[:, :cs, :], scA[:, :cs, :], AF.Exp, scale=SCALE
                            )
                            nc.scalar.activation(
                                ptB[:, :cs, :], scB[:, :cs, :], AF.Exp, scale=SCALE
                            )
                            for t in range(cs):
                                j = j0 + t
                                nc.tensor.matmul(
                                    oA[:, :],
                                    V2[:, 2 * p, j * 65:j * 65 + 128],
                                    ptA[:, t, :],
                                    start=(j == 0), stop=(j == NJ - 1),
                                )
                                nc.tensor.matmul(
                                    oB[:, :],
                                    V2[:, 2 * p + 1, j * 65:j * 65 + 128],
                                    ptB[:, t, :],
                                    start=(j == 0), stop=(j == NJ - 1),
                                )
                            j0 += cs
                        # normalize: row 64 holds the softmax denominator
                        for h, o_ps in ((2 * p, oA), (2 * p + 1, oB)):
                            dn = dn_pool.tile([1, 512], F32, tag="dn")
                            nc.vector.tensor_copy(dn[:], o_ps[64:65, :])
                            bc = bc_pool.tile([64, 512], F32, tag="bc")
                            nc.gpsimd.partition_broadcast(bc[:], dn[:], channels=64)
                            nc.vector.reciprocal(bc[:], bc[:])
                            nc.vector.tensor_mul(
                                y6[0:64, h, qs], o_ps[0:64, :], bc[:]
                            )

                    # ---------- phase 4: out-projection for this q-half ----
                    for m in range(qh * 4, (qh + 1) * 4):
                        ms = slice(m * 128, (m + 1) * 128)
                        ob = ob_pool.tile([128, D], F32, tag="ob")
                        for n0, nw in ((0, 512), (512, 256)):
                            ps = ps_out.tile([128, 512], F32, tag="outA")
                            for h in range(NH):
                                nc.tensor.matmul(
                                    ps[:, :nw],
                                    y6[:, h, ms],
                                    ww6[:, h, n0:n0 + nw],
                                    start=(h == 0), stop=(h == NH - 1),
                                )
                            nc.vector.tensor_copy(ob[:, n0:n0 + nw], ps[:, :nw])
                        nc.sync.dma_start(out[ms, :], ob[:])

    nc.finalize()  # Bacc.compile(): reg alloc + split multi-sem-waits
    return nc


_NC_CACHE = None


def make_in_maps(x, wq, bq, wk, bk, wv, ww):
    npdt = mybir.dt.np(MD)
    x = np.ascontiguousarray(np.asarray(x, dtype=np.float32))
    xT_full = np.ascontiguousarray(x[0].T).astype(npdt)  # [D, S]
    in_maps = []
    for core in range(8):
        g, c = core // NC, core % NC
        gs = slice(g * DH, (g + 1) * DH)
        in_maps.append({
            "xT": xT_full,
            "xqT": np.ascontiguousarray(xT_full[:, c * SQ:(c + 1) * SQ]),
            "wqT": np.ascontiguousarray(wq[gs, :].T).astype(npdt),
            "wkT": np.ascontiguousarray(wk[gs, :].T).astype(npdt),
            "wvT": np.ascontiguousarray(wv[gs, :].T).astype(npdt),
            "wwT": np.ascontiguousarray(ww[:, gs].T).astype(npdt),
            "bq": np.ascontiguousarray(bq[gs].reshape(NP, 128).T),
            "bk": np.ascontiguousarray(bk[gs].reshape(NP, 128).T),
        })
    return in_maps


def kernel(x, wq, bq, wk, bk, wv, bv, ww, bw):
    global _NC_CACHE
    if _NC_CACHE is None:
        _NC_CACHE = build_nc()
    nc = _NC_CACHE

    in_maps = make_in_maps(x, wq, bq, wk, bk, wv, ww)
    res = run_bass_kernel_spmd(nc, in_maps, core_ids=list(range(8)))

    const_row = (bv @ ww.T + bw).astype(np.float32)  # [768]
    out = np.empty((1, S, D), dtype=np.float32)
    for c in range(NC):
        acc = res.results[0 * NC + c]["out"] + res.results[1 * NC + c]["out"]
        out[0, c * SQ:(c + 1) * SQ, :] = acc + const_row
    return out
